# revision 19
# baseline (speedup 1.0000x reference)
"""Trainium2 Bass kernel for nn_BatchedDynamicThresholdLIF.

Reference (fp32), T=1000 sequential steps on state (B=64, N=1024):
    vp = v + (V_REST - v)/20 + x_t ; s = (vp >= th)
    th' = th + 5s - (th + 50)/100 ; v' = s ? -65 : vp

Scaled-threshold reformulation (verified on CPU vs the bit-exact
oracle: 4/65.5M spike mismatches, rel err 3.4e-3 < 2e-2 gate):
  Q = v + 50 (membrane), e = (th + 50)/0.99^t (globally scaled threshold;
  fp32 range/precision verified over all T=1000 steps, no rescale needed).
  Per step t:
    s  = [fl(e * c_t) <= Q]            c_t = fl(0.99^t)
    e += fl(d_t * s)                   d_t = fl(5/0.99^(t+1))
    Qn = fl(fl(0.95*Q) + xaQ(t+1))     xaQ = fl(x - 0.75)
    Q' = s ? b1Q(t+1) : Qn             b1Q = fl(xaQ + c15), c15 = fl(0.95f*-15)
  Q(0) = fl(c15 + xaQ(0)).

All four per-step ops run on DVE (in-order, all dependency cycles are
2 semaphore links); Pool computes bulk xaQ/b1Q per block; no ACT, no
cross-engine dependency on the recurrence. Sharding: data-parallel
over B across 8 cores (8 batch rows = 8192 state elements per core,
[128 partitions x 64 free]); T recurrence local; no cross-core comm.
"""
import numpy as np

T, B, N = 1000, 64, 1024
NCORES = 8
BS = B // NCORES            # batch rows per core
S = BS * N                  # 8192 state elements per core
P = 128                     # SBUF partitions
F = S // P                  # 64 free elements per partition
KB = 50                     # timesteps per DMA block

_nc_cache = {}


def _build():
    import concourse.bacc as bacc
    import concourse.mybir as mybir
    import concourse.tile as tile

    f32 = mybir.dt.float32
    A = mybir.AluOpType
    AF = mybir.ActivationFunctionType
    nc = bacc.Bacc(None)
    x = nc.dram_tensor("x", [T, S], f32, kind="ExternalInput")
    so = nc.dram_tensor("s", [T, S], f32, kind="ExternalOutput")
    xv = x.rearrange("t (p j) -> p t j", p=P)
    sv = so.rearrange("t (p j) -> p t j", p=P)
    nblk = T // KB
    c15 = float(np.float32(np.float32(0.95) * np.float32(-15.0)))
    # global scaling frame: e = (th + 50) / 0.99^t over the whole run
    # (fp32 range/precision verified: 4/65.5M mismatches, same as blocked)
    ck = [float(np.float32(np.float64(0.99) ** t)) for t in range(T)]
    dk = [float(np.float32(5.0 / (np.float64(0.99) ** (t + 1)))) for t in range(T)]

    # DMA blocking: graduated ramp (each block's DMA + ACT prep hides
    # under the previous block's compute) then steady 50-step blocks.
    # Block sizes don't affect numerics (e-scaling frame is global).
    sizes = [1, 2, 8, 16, 32, 41] + [75] * 12
    blocks = []
    t0 = 0
    for L in sizes:
        blocks.append((t0, L))
        t0 += L
    assert t0 == T
    # last block's output DMA is split at these in-block positions so the
    # big transfers overlap the final steps
    TAIL_SPLITS = [60, 73]

    with tile.TileContext(nc) as tc:
        with tc.tile_pool(name="st", bufs=1) as stp, \
             tc.tile_pool(name="xp", bufs=2) as xp, \
             tc.tile_pool(name="xa", bufs=2) as xap, \
             tc.tile_pool(name="b1", bufs=2) as b1p, \
             tc.tile_pool(name="sp", bufs=2) as sp:
            qA = stp.tile([P, F], f32, name="qA")
            qB = stp.tile([P, F], f32, name="qB")
            e = stp.tile([P, F], f32, name="e")
            def fetch(bi):
                # Load L+1 rows (one overlap row) so step t0+L-1 reads
                # xa(t0+L) from this block's own tile — no cross-tile
                # dependency at block boundaries.
                t0, L = blocks[bi]
                R = min(L + 1, T - t0)
                xb = xp.tile([P, R, F], f32, name="xb", tag="xb")
                nc.sync.dma_start(out=xb, in_=xv[:, t0:t0 + R, :])
                xa = xap.tile([P, R, F], f32, name="xa", tag="xa")
                nc.scalar.activation(xa, xb, AF.Copy, bias=-0.75, scale=1.0)
                b1 = b1p.tile([P, R, F], f32, name="b1", tag="b1")
                nc.scalar.activation(b1, xa, AF.Copy, bias=c15, scale=1.0)
                return xa, b1

            xa_cur, b1_cur = fetch(0)
            nc.vector.memset(e, 0.0)
            # Q(0) = fl(xaQ(0) + c15)
            nc.vector.tensor_scalar(qA, xa_cur[:, 0, :], c15, None, A.add)

            t = 0
            for bi, (t0, L) in enumerate(blocks):
                nxt = fetch(bi + 1) if bi + 1 < len(blocks) else (None, None)
                xa_nxt, b1_nxt = nxt
                sb = sp.tile([P, L, F], f32, name="sb", tag="sb")
                for k in range(L):
                    q_cur, q_nxt = (qA, qB) if t % 2 == 0 else (qB, qA)
                    st_ = sb[:, k, :]
                    last = t == T - 1
                    if not last:
                        xan, b1n = xa_cur[:, k + 1, :], b1_cur[:, k + 1, :]
                        # membrane candidate first: only depends on cp(t-1)
                        nc.vector.scalar_tensor_tensor(
                            q_nxt, q_cur, 0.95, xan, A.mult, A.add)
                    # s = (e * c_t) is_le Q
                    nc.vector.scalar_tensor_tensor(
                        st_, e, ck[t], q_cur, A.mult, A.is_le)
                    if not last:
                        nc.vector.copy_predicated(
                            q_nxt, st_.bitcast(mybir.dt.uint32), b1n)
                        nc.vector.scalar_tensor_tensor(
                            e, st_, dk[t], e, A.mult, A.add)
                    t += 1
                    if bi == len(blocks) - 1 and k + 1 in TAIL_SPLITS:
                        lo = ([0] + TAIL_SPLITS)[TAIL_SPLITS.index(k + 1)]
                        nc.sync.dma_start(
                            out=sv[:, t0 + lo:t0 + k + 1, :],
                            in_=sb[:, lo:k + 1, :])
                if bi == len(blocks) - 1:
                    lo = TAIL_SPLITS[-1]
                    nc.sync.dma_start(out=sv[:, t0 + lo:t0 + L, :],
                                      in_=sb[:, lo:, :])
                else:
                    nc.sync.dma_start(out=sv[:, t0:t0 + L, :], in_=sb)
                xa_cur, b1_cur = xa_nxt, b1_nxt
    nc.compile()
    return nc


def _get_nc():
    if "nc" not in _nc_cache:
        _nc_cache["nc"] = _build()
    return _nc_cache["nc"]


def kernel(weighted_input: np.ndarray) -> np.ndarray:
    from concourse.bass_utils import run_bass_kernel_spmd

    x = np.ascontiguousarray(np.asarray(weighted_input, dtype=np.float32))
    assert x.shape == (T, B, N), x.shape
    nc = _get_nc()
    in_maps = []
    for c in range(NCORES):
        xc = np.ascontiguousarray(x[:, c * BS:(c + 1) * BS, :].reshape(T, S))
        in_maps.append({"x": xc})
    res = run_bass_kernel_spmd(nc, in_maps, core_ids=list(range(NCORES)))
    out = np.empty((T, B, N), np.float32)
    for c in range(NCORES):
        out[:, c * BS:(c + 1) * BS, :] = res.results[c]["s"].reshape(T, BS, N)
    return out


if __name__ == "__main__":
    x = np.random.default_rng(0).standard_normal((T, B, N)).astype(np.float32) * 3.0
    s = kernel(x)
    print("spike rate:", s.mean())
